# revision 8
# baseline (speedup 1.0000x reference)
"""Trainium2 Bass kernel for nn_MetaphorModel (masked segment-mean pool +
tiny linear classifier + CE loss).

Strategy (pure data parallel, 8 NeuronCores):
  - Shard batch B=256 across 8 cores (32 samples/core).
  - Only ~half the S=512 token rows are masked-in. The device gathers
    just those rows with indirect DMA whose inline CCE ALU *accumulates*
    (compute_op=add) into per-(sample-slot) SBUF lines, so the masked
    sum happens inside the DMA datapath at line rate — no per-row
    matmul streaming at all.
  - The mask's row set is decomposed on host into runs of 4/2/1
    consecutive rows; each indirect DMA moves 128 runs (one per
    partition). Partitions are statically assigned to samples
    (proportional to per-sample run counts), so each SBUF partition
    accumulates a partial sum of exactly one sample's rows.
  - A single small fp32 matmul pass (7 x 2 accumulating matmuls with a
    [128, 32] per-slot indicator carrying 1/count) folds the partials
    into pooled [32, 768] in PSUM; the 768->2 classifier then runs on
    the vector engine; logits [32, 2] DMA out per core.
  - Host computes the scalar CE loss from the gathered [256, 2] logits
    (trivial epilogue) and returns (loss, logits) like the reference.

All host-side prep derives only from the tiny [256, 512] bool mask; the
384 MB of hidden states is touched exclusively by the device.
"""

import numpy as np

_B, _S, _D, _C = 256, 512, 768, 2
_NCORES = 8
_BPC = _B // _NCORES        # samples per core = 32
_P = 128                    # SBUF partitions
_SIZES = (4, 2, 1)          # gather run lengths (rows per descriptor)
_ZPAD = 4                   # zero rows appended to the h shard for pads

LAST_RESULTS = None


def _build_program(n_dmas):
    """n_dmas: dict size->number of gather DMAs of that run size."""
    from contextlib import ExitStack

    import concourse.bacc as bacc
    import concourse.bass as bass
    import concourse.mybir as mybir
    import concourse.tile as tile

    f32 = mybir.dt.float32
    i32 = mybir.dt.int32
    nc = bacc.Bacc("TRN2", target_bir_lowering=False, debug=False,
                   num_devices=_NCORES)

    h_d = nc.dram_tensor("h", [_BPC * _S + _ZPAD, _D], f32,
                         kind="ExternalInput")
    ix_d = {u: nc.dram_tensor(f"ix{u}", [_P, max(n_dmas[u], 1)], i32,
                              kind="ExternalInput") for u in _SIZES}
    cw_d = nc.dram_tensor("cw", [_P, len(_SIZES) * _BPC], f32,
                          kind="ExternalInput")
    w_d = nc.dram_tensor("wrep", [_BPC, _C * _D], f32, kind="ExternalInput")
    b_d = nc.dram_tensor("brep", [_BPC, _C], f32, kind="ExternalInput")
    lg_d = nc.dram_tensor("logits", [_BPC, _C], f32, kind="ExternalOutput")

    with tile.TileContext(nc) as tc, ExitStack() as ctx:
        cpool = ctx.enter_context(tc.tile_pool(name="const", bufs=1))
        apool = ctx.enter_context(tc.tile_pool(name="acc", bufs=1))
        pspool = ctx.enter_context(tc.tile_pool(name="ps", bufs=1,
                                                space="PSUM"))
        epool = ctx.enter_context(tc.tile_pool(name="ep", bufs=1))

        ix_sb = {}
        for u in _SIZES:
            ix_sb[u] = cpool.tile([_P, max(n_dmas[u], 1)], i32,
                                  name=f"ixsb{u}")
            nc.sync.dma_start(ix_sb[u][:], ix_d[u].ap())
        cw_sb = cpool.tile([_P, len(_SIZES) * _BPC], f32)
        nc.sync.dma_start(cw_sb[:], cw_d.ap())
        wrep = cpool.tile([_BPC, _C * _D], f32)
        nc.sync.dma_start(wrep[:], w_d.ap())
        brep = cpool.tile([_BPC, _C], f32)
        nc.sync.dma_start(brep[:], b_d.ap())

        acc = {}
        for u in _SIZES:
            acc[u] = apool.tile([_P, u * _D], f32, tag=f"acc{u}",
                                name=f"acc{u}")
            nc.vector.memset(acc[u][:], 0.0)

        # interleave gather issue across run sizes (round robin) so SWDGE
        # emission of short transfers hides behind long drains
        order = []
        cnt = {u: 0 for u in _SIZES}
        while any(cnt[u] < n_dmas[u] for u in _SIZES):
            for u in _SIZES:
                if cnt[u] < n_dmas[u]:
                    order.append((u, cnt[u]))
                    cnt[u] += 1
        for u, g in order:
            nc.gpsimd.indirect_dma_start(
                out=acc[u][:], out_offset=None, in_=h_d.ap()[:],
                in_offset=bass.IndirectOffsetOnAxis(
                    ap=ix_sb[u][:, g:g + 1], axis=0),
                compute_op=mybir.AluOpType.add)

        # combine slot partials -> pooled [32, 768] (PSUM accumulate)
        pooled = pspool.tile([_BPC, _D], f32)
        groups = [(u, g) for u in _SIZES for g in range(u)]
        for i, (u, g) in enumerate(groups):
            lhsT = cw_sb[:, _SIZES.index(u) * _BPC:
                         (_SIZES.index(u) + 1) * _BPC]
            first, last = (i == 0), (i == len(groups) - 1)
            nc.tensor.matmul(pooled[:, 0:512], lhsT,
                             acc[u][:, g * _D:g * _D + 512],
                             start=first, stop=last)
            nc.tensor.matmul(pooled[:, 512:_D], lhsT,
                             acc[u][:, g * _D + 512:(g + 1) * _D],
                             start=first, stop=last)

        lg = epool.tile([_BPC, _C], f32)
        for ci in range(_C):
            prod = epool.tile([_BPC, _D], f32, tag=f"prod{ci}")
            nc.vector.tensor_mul(prod[:], pooled[:],
                                 wrep[:, ci * _D:(ci + 1) * _D])
            nc.vector.reduce_sum(lg[:, ci:ci + 1], prod[:],
                                 axis=mybir.AxisListType.X)
        nc.vector.tensor_add(lg[:], lg[:], brep[:])
        nc.sync.dma_start(lg_d.ap(), lg[:])

    nc.compile()
    return nc


def _decompose_runs(mask_row):
    """Masked positions of one sample -> dict size -> list of run starts."""
    s = np.flatnonzero(mask_row)
    out = {u: [] for u in _SIZES}
    i = 0
    n = len(s)
    while i < n:
        j = i
        while j + 1 < n and s[j + 1] == s[j] + 1:
            j += 1
        L = j - i + 1  # maximal run s[i..j]
        pos = s[i]
        for u in _SIZES:
            while L >= u:
                out[u].append(pos)
                pos += u
                L -= u
        i = j + 1
    return out


def _min_ndma(cnts, n_slots=_P):
    """Minimal n with sum_b ceil(cnt_b/n) <= n_slots."""
    total = int(sum(cnts))
    if total == 0:
        return 0
    n = max(1, -(-total // n_slots))
    while True:
        if sum(-(-c // n) for c in cnts) <= n_slots:
            return n
        n += 1


def _prep(h, mask, W, bias):
    """Returns (n_dmas, in_maps)."""
    maskf = mask.astype(np.float32)
    counts = maskf.sum(axis=1)                      # [B]

    runs = [_decompose_runs(mask[b]) for b in range(_B)]
    cnts = {u: np.array([len(runs[b][u]) for b in range(_B)]) for u in _SIZES}

    n_dmas = {}
    for u in _SIZES:
        n_dmas[u] = max(
            _min_ndma(cnts[u][c * _BPC:(c + 1) * _BPC]) for c in range(_NCORES))

    wrep_np = np.ascontiguousarray(
        np.broadcast_to(W.reshape(1, _C * _D), (_BPC, _C * _D)))
    brep_np = np.ascontiguousarray(
        np.broadcast_to(bias.reshape(1, _C), (_BPC, _C)))

    pad_row = _BPC * _S  # first appended zero row

    in_maps = []
    for core in range(_NCORES):
        b0 = core * _BPC
        hc = np.empty((_BPC * _S + _ZPAD, _D), np.float32)
        hc[:_BPC * _S] = h[b0:b0 + _BPC].reshape(_BPC * _S, _D)
        hc[_BPC * _S:] = 0.0

        im = {"h": hc, "wrep": wrep_np, "brep": brep_np}
        cw_np = np.zeros((_P, len(_SIZES) * _BPC), np.float32)
        for ui, u in enumerate(_SIZES):
            n = n_dmas[u]
            ix_np = np.full((_P, max(n, 1)), pad_row, np.int32)
            if n > 0:
                slot = 0
                for b in range(_BPC):
                    starts = runs[b0 + b][u]
                    nslots = -(-len(starts) // n) if starts else 0
                    for q in range(nslots):
                        chunk = starts[q * n:(q + 1) * n]
                        ix_np[slot, :len(chunk)] = \
                            (np.asarray(chunk) + b * _S).astype(np.int32)
                        cw_np[slot, ui * _BPC + b] = 1.0 / counts[b0 + b]
                        slot += 1
                assert slot <= _P
            im[f"ix{u}"] = np.ascontiguousarray(ix_np)
        im["cw"] = cw_np
        in_maps.append(im)
    return n_dmas, in_maps


def kernel(last_hidden_state, metaphor_mask, labels, classifier_w,
           classifier_b):
    global LAST_RESULTS
    from concourse.bass_utils import run_bass_kernel_spmd

    h = np.asarray(last_hidden_state, dtype=np.float32)
    mask = np.asarray(metaphor_mask).astype(bool)
    labels = np.asarray(labels)
    W = np.asarray(classifier_w, dtype=np.float32)
    bias = np.asarray(classifier_b, dtype=np.float32)

    assert h.shape == (_B, _S, _D) and W.shape == (_C, _D)

    n_dmas, in_maps = _prep(h, mask, W, bias)
    nc = _build_program(n_dmas)

    res = run_bass_kernel_spmd(nc, in_maps, core_ids=list(range(_NCORES)))
    LAST_RESULTS = res
    logits = np.concatenate([r["logits"] for r in res.results], axis=0)

    # Host epilogue: CE loss (mean reduction) over the tiny [256, 2] logits.
    lg64 = logits.astype(np.float64)
    m = lg64.max(axis=1, keepdims=True)
    lse = (m[:, 0] + np.log(np.exp(lg64 - m).sum(axis=1)))
    nll = lse - lg64[np.arange(_B), labels.astype(np.int64)]
    loss = np.float32(nll.mean())
    return loss, logits


# revision 9
# speedup vs baseline: 2.1090x; 2.1090x over previous
"""Trainium2 Bass kernel for nn_MetaphorModel (masked segment-mean pool +
tiny linear classifier + CE loss).

Strategy (pure data parallel, 8 NeuronCores):
  - Shard batch B=256 across 8 cores (32 samples/core).
  - Only ~half the S=512 token rows are masked-in; the device gathers
    just those rows from HBM with indirect (gather) DMA, cutting HBM
    traffic ~2x vs a dense read. The mask's row set is decomposed on
    host into runs of 4/2/1 consecutive rows; each indirect DMA moves
    128 runs (one per SBUF partition). Longer runs mean fewer, larger
    DMA descriptors. Issue order round-robins across run sizes so SWDGE
    descriptor emission stays ahead of the SDMA engines.
  - The masked mean-pool is a matmul over the gathered rows: for each
    128-row chunk, lhsT is a [128, 32] "indicator" matrix whose column b
    holds 1/count[b] at rows belonging to sample b (0 elsewhere, 0 for
    pad rows). PSUM accumulates pooled [32, 768] across all chunks.
  - Classifier (768 -> 2) runs on the vector engine as two
    multiply+reduce ops against replicated weight rows, plus bias.
    Logits [32, 2] DMA'd out per core.
  - Host computes the scalar CE loss from the gathered [256, 2] logits
    (trivial epilogue) and returns (loss, logits) like the reference.

All host-side prep derives only from the tiny [256, 512] bool mask; the
384 MB of hidden states is touched exclusively by the device.
"""

import numpy as np

_B, _S, _D, _C = 256, 512, 768, 2
_NCORES = 8
_BPC = _B // _NCORES        # samples per core = 32
_P = 128                    # SBUF partitions
_SIZES = (4, 2, 1)          # gather run lengths (rows per descriptor)
_IND_SPLIT = 4              # indicator table loaded as this many tiles

LAST_RESULTS = None


def _issue_order(n_dmas):
    """Round-robin DMA issue order across run sizes: [(u, g), ...]."""
    order = []
    cnt = {u: 0 for u in _SIZES}
    while any(cnt[u] < n_dmas[u] for u in _SIZES):
        for u in _SIZES:
            if cnt[u] < n_dmas[u]:
                order.append((u, cnt[u]))
                cnt[u] += 1
    return order


def _build_program(n_dmas):
    """n_dmas: dict size -> number of gather DMAs of that run size."""
    from contextlib import ExitStack

    import concourse.bacc as bacc
    import concourse.bass as bass
    import concourse.mybir as mybir
    import concourse.tile as tile

    f32 = mybir.dt.float32
    i32 = mybir.dt.int32
    order = _issue_order(n_dmas)
    nch = sum(u for u, _ in order)
    csz = -(-nch // _IND_SPLIT)  # chunks per indicator tile

    nc = bacc.Bacc("TRN2", target_bir_lowering=False, debug=False,
                   num_devices=_NCORES)

    h_d = nc.dram_tensor("h", [_BPC * _S, _D], f32, kind="ExternalInput")
    ix_d = {u: nc.dram_tensor(f"ix{u}", [_P, max(n_dmas[u], 1)], i32,
                              kind="ExternalInput") for u in _SIZES}
    ind_d = nc.dram_tensor("ind", [_P, nch * _BPC], f32,
                           kind="ExternalInput")
    w_d = nc.dram_tensor("wrep", [_BPC, _C * _D], f32, kind="ExternalInput")
    b_d = nc.dram_tensor("brep", [_BPC, _C], f32, kind="ExternalInput")
    lg_d = nc.dram_tensor("logits", [_BPC, _C], f32, kind="ExternalOutput")

    with tile.TileContext(nc) as tc, ExitStack() as ctx:
        pools = {
            4: ctx.enter_context(tc.tile_pool(name="h4", bufs=6)),
            2: ctx.enter_context(tc.tile_pool(name="h2", bufs=8)),
            1: ctx.enter_context(tc.tile_pool(name="h1", bufs=10)),
        }
        cpool = ctx.enter_context(tc.tile_pool(name="const", bufs=1))
        pspool = ctx.enter_context(tc.tile_pool(name="ps", bufs=1,
                                                space="PSUM"))
        epool = ctx.enter_context(tc.tile_pool(name="ep", bufs=1))

        ix_sb = {}
        for u in _SIZES:
            ix_sb[u] = cpool.tile([_P, max(n_dmas[u], 1)], i32,
                                  name=f"ixsb{u}")
            nc.sync.dma_start(ix_sb[u][:], ix_d[u].ap())
        ind_sb = []
        for t in range(_IND_SPLIT):
            lo = t * csz * _BPC
            hi = min(nch, (t + 1) * csz) * _BPC
            tl = cpool.tile([_P, max(hi - lo, _BPC)], f32, name=f"indsb{t}")
            if hi > lo:
                nc.sync.dma_start(tl[:, :hi - lo], ind_d.ap()[:, lo:hi])
            ind_sb.append(tl)
        wrep = cpool.tile([_BPC, _C * _D], f32)
        nc.sync.dma_start(wrep[:], w_d.ap())
        brep = cpool.tile([_BPC, _C], f32)
        nc.sync.dma_start(brep[:], b_d.ap())

        def ind_col(k):
            t, r = k // csz, k % csz
            return ind_sb[t][:, r * _BPC:(r + 1) * _BPC]

        pooled = pspool.tile([_BPC, _D], f32)
        k = 0
        for u, g in order:
            ht = pools[u].tile([_P, u * _D], f32, name=f"ht{u}",
                               tag=f"ht{u}")
            nc.gpsimd.indirect_dma_start(
                out=ht[:], out_offset=None, in_=h_d.ap()[:],
                in_offset=bass.IndirectOffsetOnAxis(
                    ap=ix_sb[u][:, g:g + 1], axis=0))
            for c in range(u):
                lhsT = ind_col(k)
                first, last = (k == 0), (k == nch - 1)
                nc.tensor.matmul(pooled[:, 0:512], lhsT,
                                 ht[:, c * _D:c * _D + 512],
                                 start=first, stop=last)
                nc.tensor.matmul(pooled[:, 512:_D], lhsT,
                                 ht[:, c * _D + 512:(c + 1) * _D],
                                 start=first, stop=last)
                k += 1

        lg = epool.tile([_BPC, _C], f32)
        for ci in range(_C):
            prod = epool.tile([_BPC, _D], f32, tag=f"prod{ci}")
            nc.vector.tensor_mul(prod[:], pooled[:],
                                 wrep[:, ci * _D:(ci + 1) * _D])
            nc.vector.reduce_sum(lg[:, ci:ci + 1], prod[:],
                                 axis=mybir.AxisListType.X)
        nc.vector.tensor_add(lg[:], lg[:], brep[:])
        nc.sync.dma_start(lg_d.ap(), lg[:])

    nc.compile()
    return nc


def _decompose_runs(mask_row):
    """Masked positions of one sample -> dict size -> list of run starts."""
    s = np.flatnonzero(mask_row)
    out = {u: [] for u in _SIZES}
    i = 0
    n = len(s)
    while i < n:
        j = i
        while j + 1 < n and s[j + 1] == s[j] + 1:
            j += 1
        L = j - i + 1  # maximal run s[i..j]
        pos = s[i]
        for u in _SIZES:
            while L >= u:
                out[u].append(pos)
                pos += u
                L -= u
        i = j + 1
    return out


def _prep(h, mask, W, bias):
    """Returns (n_dmas, in_maps)."""
    maskf = mask.astype(np.float32)
    counts = maskf.sum(axis=1)                      # [B]

    runs = [_decompose_runs(mask[b]) for b in range(_B)]

    # per-core dense unit lists per size: (row, owner) in sample order
    units = {}
    for core in range(_NCORES):
        b0 = core * _BPC
        for u in _SIZES:
            rows, owners = [], []
            for b in range(_BPC):
                for st in runs[b0 + b][u]:
                    rows.append(b * _S + st)
                    owners.append(b)
            units[(core, u)] = (np.asarray(rows, np.int32),
                               np.asarray(owners, np.int64))

    n_dmas = {u: max(1, max(-(-len(units[(c, u)][0]) // _P)
                            for c in range(_NCORES))) for u in _SIZES}
    order = _issue_order(n_dmas)
    nch = sum(u for u, _ in order)

    # chunk base index of each DMA in issue order
    cbase = {}
    k = 0
    for u, g in order:
        cbase[(u, g)] = k
        k += u

    wrep_np = np.ascontiguousarray(
        np.broadcast_to(W.reshape(1, _C * _D), (_BPC, _C * _D)))
    brep_np = np.ascontiguousarray(
        np.broadcast_to(bias.reshape(1, _C), (_BPC, _C)))

    in_maps = []
    for core in range(_NCORES):
        b0 = core * _BPC
        hc = np.ascontiguousarray(h[b0:b0 + _BPC].reshape(_BPC * _S, _D))
        im = {"h": hc, "wrep": wrep_np, "brep": brep_np}
        ind_np = np.zeros((_P, nch * _BPC), np.float32)
        for u in _SIZES:
            rows, owners = units[(core, u)]
            n = n_dmas[u]
            ix_np = np.zeros((_P, n), np.int32)
            nu = len(rows)
            if nu:
                w = (1.0 / counts[b0 + owners]).astype(np.float32)
                for i in range(nu):
                    g, p = divmod(i, _P)
                    ix_np[p, g] = rows[i]
                    kb = cbase[(u, g)]
                    for c in range(u):
                        ind_np[p, (kb + c) * _BPC + owners[i]] = w[i]
            im[f"ix{u}"] = np.ascontiguousarray(ix_np)
        im["ind"] = ind_np
        in_maps.append(im)
    return n_dmas, in_maps


def kernel(last_hidden_state, metaphor_mask, labels, classifier_w,
           classifier_b):
    global LAST_RESULTS
    from concourse.bass_utils import run_bass_kernel_spmd

    h = np.asarray(last_hidden_state, dtype=np.float32)
    mask = np.asarray(metaphor_mask).astype(bool)
    labels = np.asarray(labels)
    W = np.asarray(classifier_w, dtype=np.float32)
    bias = np.asarray(classifier_b, dtype=np.float32)

    assert h.shape == (_B, _S, _D) and W.shape == (_C, _D)

    n_dmas, in_maps = _prep(h, mask, W, bias)
    nc = _build_program(n_dmas)

    res = run_bass_kernel_spmd(nc, in_maps, core_ids=list(range(_NCORES)))
    LAST_RESULTS = res
    logits = np.concatenate([r["logits"] for r in res.results], axis=0)

    # Host epilogue: CE loss (mean reduction) over the tiny [256, 2] logits.
    lg64 = logits.astype(np.float64)
    m = lg64.max(axis=1, keepdims=True)
    lse = (m[:, 0] + np.log(np.exp(lg64 - m).sum(axis=1)))
    nll = lse - lg64[np.arange(_B), labels.astype(np.int64)]
    loss = np.float32(nll.mean())
    return loss, logits


# revision 10
# speedup vs baseline: 2.4263x; 1.1505x over previous
"""Trainium2 Bass kernel for nn_MetaphorModel (masked segment-mean pool +
tiny linear classifier + CE loss).

Strategy (pure data parallel, 8 NeuronCores):
  - Shard batch B=256 across 8 cores (32 samples/core).
  - Only ~half the S=512 token rows are masked-in; the device gathers
    just those rows from HBM with indirect (gather) DMA, cutting HBM
    traffic ~2x vs a dense read. The mask's row set is decomposed on
    host into runs of 4/2/1 consecutive rows; each indirect DMA moves
    128 runs (one per SBUF partition). Longer runs mean fewer, larger
    DMA descriptors. Issue order round-robins across run sizes so SWDGE
    descriptor emission stays ahead of the SDMA engines.
  - The masked mean-pool is a matmul over the gathered rows: for each
    128-row chunk, lhsT is a [128, 32] "indicator" matrix whose column b
    holds 1/count[b] at rows belonging to sample b (0 elsewhere, 0 for
    pad rows). PSUM accumulates pooled [32, 768] across all chunks.
  - Classifier (768 -> 2) runs on the vector engine as two
    multiply+reduce ops against replicated weight rows, plus bias.
    Logits [32, 2] DMA'd out per core.
  - Host computes the scalar CE loss from the gathered [256, 2] logits
    (trivial epilogue) and returns (loss, logits) like the reference.

All host-side prep derives only from the tiny [256, 512] bool mask; the
384 MB of hidden states is touched exclusively by the device.
"""

import numpy as np

_B, _S, _D, _C = 256, 512, 768, 2
_NCORES = 8
_BPC = _B // _NCORES        # samples per core = 32
_P = 128                    # SBUF partitions
_SIZES = (4, 2, 1)          # gather run lengths (rows per descriptor)
_IND_SPLIT = 4              # indicator table loaded as this many tiles

LAST_RESULTS = None


def _issue_order(n_dmas):
    """Round-robin DMA issue order across run sizes: [(u, g), ...]."""
    order = []
    cnt = {u: 0 for u in _SIZES}
    while any(cnt[u] < n_dmas[u] for u in _SIZES):
        for u in _SIZES:
            if cnt[u] < n_dmas[u]:
                order.append((u, cnt[u]))
                cnt[u] += 1
    return order


def _build_program(n_dmas):
    """n_dmas: dict size -> number of gather DMAs of that run size."""
    from contextlib import ExitStack

    import concourse.bacc as bacc
    import concourse.bass as bass
    import concourse.mybir as mybir
    import concourse.tile as tile

    f32 = mybir.dt.float32
    f32r = mybir.dt.float32r
    i32 = mybir.dt.int32
    order = _issue_order(n_dmas)
    nch = sum(u for u, _ in order)
    csz = -(-nch // _IND_SPLIT)  # chunks per indicator tile

    nc = bacc.Bacc("TRN2", target_bir_lowering=False, debug=False,
                   num_devices=_NCORES)

    h_d = nc.dram_tensor("h", [_BPC * _S, _D], f32r, kind="ExternalInput")
    ix_d = {u: nc.dram_tensor(f"ix{u}", [_P, max(n_dmas[u], 1)], i32,
                              kind="ExternalInput") for u in _SIZES}
    ind_d = nc.dram_tensor("ind", [_P, nch * _BPC], f32r,
                           kind="ExternalInput")
    w_d = nc.dram_tensor("wrep", [_BPC, _C * _D], f32, kind="ExternalInput")
    b_d = nc.dram_tensor("brep", [_BPC, _C], f32, kind="ExternalInput")
    lg_d = nc.dram_tensor("logits", [_BPC, _C], f32, kind="ExternalOutput")

    with tile.TileContext(nc) as tc, ExitStack() as ctx:
        pools = {
            4: ctx.enter_context(tc.tile_pool(name="h4", bufs=6)),
            2: ctx.enter_context(tc.tile_pool(name="h2", bufs=8)),
            1: ctx.enter_context(tc.tile_pool(name="h1", bufs=10)),
        }
        cpool = ctx.enter_context(tc.tile_pool(name="const", bufs=1))
        pspool = ctx.enter_context(tc.tile_pool(name="ps", bufs=1,
                                                space="PSUM"))
        epool = ctx.enter_context(tc.tile_pool(name="ep", bufs=1))

        ix_sb = {}
        for u in _SIZES:
            ix_sb[u] = cpool.tile([_P, max(n_dmas[u], 1)], i32,
                                  name=f"ixsb{u}")
            nc.sync.dma_start(ix_sb[u][:], ix_d[u].ap())
        ind_sb = []
        for t in range(_IND_SPLIT):
            lo = t * csz * _BPC
            hi = min(nch, (t + 1) * csz) * _BPC
            tl = cpool.tile([_P, max(hi - lo, _BPC)], f32r, name=f"indsb{t}")
            if hi > lo:
                nc.sync.dma_start(tl[:, :hi - lo], ind_d.ap()[:, lo:hi])
            ind_sb.append(tl)
        wrep = cpool.tile([_BPC, _C * _D], f32)
        nc.sync.dma_start(wrep[:], w_d.ap())
        brep = cpool.tile([_BPC, _C], f32)
        nc.sync.dma_start(brep[:], b_d.ap())

        def ind_col(k):
            t, r = k // csz, k % csz
            return ind_sb[t][:, r * _BPC:(r + 1) * _BPC]

        pooled = pspool.tile([_BPC, _D], f32)
        k = 0
        for u, g in order:
            ht = pools[u].tile([_P, u * _D], f32r, name=f"ht{u}",
                               tag=f"ht{u}")
            nc.gpsimd.indirect_dma_start(
                out=ht[:], out_offset=None, in_=h_d.ap()[:],
                in_offset=bass.IndirectOffsetOnAxis(
                    ap=ix_sb[u][:, g:g + 1], axis=0))
            for c in range(u):
                lhsT = ind_col(k)
                first, last = (k == 0), (k == nch - 1)
                nc.tensor.matmul(pooled[:, 0:512], lhsT,
                                 ht[:, c * _D:c * _D + 512],
                                 start=first, stop=last)
                nc.tensor.matmul(pooled[:, 512:_D], lhsT,
                                 ht[:, c * _D + 512:(c + 1) * _D],
                                 start=first, stop=last)
                k += 1

        lg = epool.tile([_BPC, _C], f32)
        for ci in range(_C):
            prod = epool.tile([_BPC, _D], f32, tag=f"prod{ci}")
            nc.vector.tensor_mul(prod[:], pooled[:],
                                 wrep[:, ci * _D:(ci + 1) * _D])
            nc.vector.reduce_sum(lg[:, ci:ci + 1], prod[:],
                                 axis=mybir.AxisListType.X)
        nc.vector.tensor_add(lg[:], lg[:], brep[:])
        nc.sync.dma_start(lg_d.ap(), lg[:])

    nc.compile()
    return nc


def _decompose_runs(mask_row):
    """Masked positions of one sample -> dict size -> list of run starts."""
    s = np.flatnonzero(mask_row)
    out = {u: [] for u in _SIZES}
    i = 0
    n = len(s)
    while i < n:
        j = i
        while j + 1 < n and s[j + 1] == s[j] + 1:
            j += 1
        L = j - i + 1  # maximal run s[i..j]
        pos = s[i]
        for u in _SIZES:
            while L >= u:
                out[u].append(pos)
                pos += u
                L -= u
        i = j + 1
    return out


def _prep(h, mask, W, bias):
    """Returns (n_dmas, in_maps)."""
    maskf = mask.astype(np.float32)
    counts = maskf.sum(axis=1)                      # [B]

    runs = [_decompose_runs(mask[b]) for b in range(_B)]

    # per-core dense unit lists per size: (row, owner) in sample order
    units = {}
    for core in range(_NCORES):
        b0 = core * _BPC
        for u in _SIZES:
            rows, owners = [], []
            for b in range(_BPC):
                for st in runs[b0 + b][u]:
                    rows.append(b * _S + st)
                    owners.append(b)
            units[(core, u)] = (np.asarray(rows, np.int32),
                               np.asarray(owners, np.int64))

    n_dmas = {u: max(1, max(-(-len(units[(c, u)][0]) // _P)
                            for c in range(_NCORES))) for u in _SIZES}
    order = _issue_order(n_dmas)
    nch = sum(u for u, _ in order)

    # chunk base index of each DMA in issue order
    cbase = {}
    k = 0
    for u, g in order:
        cbase[(u, g)] = k
        k += u

    wrep_np = np.ascontiguousarray(
        np.broadcast_to(W.reshape(1, _C * _D), (_BPC, _C * _D)))
    brep_np = np.ascontiguousarray(
        np.broadcast_to(bias.reshape(1, _C), (_BPC, _C)))

    in_maps = []
    for core in range(_NCORES):
        b0 = core * _BPC
        hc = np.ascontiguousarray(h[b0:b0 + _BPC].reshape(_BPC * _S, _D))
        im = {"h": hc, "wrep": wrep_np, "brep": brep_np}
        ind_np = np.zeros((_P, nch * _BPC), np.float32)
        for u in _SIZES:
            rows, owners = units[(core, u)]
            n = n_dmas[u]
            ix_np = np.zeros((_P, n), np.int32)
            nu = len(rows)
            if nu:
                w = (1.0 / counts[b0 + owners]).astype(np.float32)
                for i in range(nu):
                    g, p = divmod(i, _P)
                    ix_np[p, g] = rows[i]
                    kb = cbase[(u, g)]
                    for c in range(u):
                        ind_np[p, (kb + c) * _BPC + owners[i]] = w[i]
            im[f"ix{u}"] = np.ascontiguousarray(ix_np)
        im["ind"] = ind_np
        in_maps.append(im)
    return n_dmas, in_maps


def kernel(last_hidden_state, metaphor_mask, labels, classifier_w,
           classifier_b):
    global LAST_RESULTS
    from concourse.bass_utils import run_bass_kernel_spmd

    h = np.asarray(last_hidden_state, dtype=np.float32)
    mask = np.asarray(metaphor_mask).astype(bool)
    labels = np.asarray(labels)
    W = np.asarray(classifier_w, dtype=np.float32)
    bias = np.asarray(classifier_b, dtype=np.float32)

    assert h.shape == (_B, _S, _D) and W.shape == (_C, _D)

    n_dmas, in_maps = _prep(h, mask, W, bias)
    nc = _build_program(n_dmas)

    res = run_bass_kernel_spmd(nc, in_maps, core_ids=list(range(_NCORES)))
    LAST_RESULTS = res
    logits = np.concatenate([r["logits"] for r in res.results], axis=0)

    # Host epilogue: CE loss (mean reduction) over the tiny [256, 2] logits.
    lg64 = logits.astype(np.float64)
    m = lg64.max(axis=1, keepdims=True)
    lse = (m[:, 0] + np.log(np.exp(lg64 - m).sum(axis=1)))
    nll = lse - lg64[np.arange(_B), labels.astype(np.int64)]
    loss = np.float32(nll.mean())
    return loss, logits


# revision 11
# speedup vs baseline: 2.6875x; 1.1076x over previous
"""Trainium2 Bass kernel for nn_MetaphorModel (masked segment-mean pool +
tiny linear classifier + CE loss).

Strategy (pure data parallel, 8 NeuronCores):
  - Shard batch B=256 across 8 cores (32 samples/core).
  - Only ~half the S=512 token rows are masked-in; the device gathers
    just those rows from HBM with indirect (gather) DMA, cutting HBM
    traffic ~2x vs a dense read. The mask's row set is decomposed on
    host into runs of 4/2/1 consecutive rows; each indirect DMA moves
    128 runs (one per SBUF partition). Longer runs mean fewer, larger
    DMA descriptors. Issue order round-robins across run sizes so SWDGE
    descriptor emission stays ahead of the SDMA engines.
  - The masked mean-pool is a matmul over the gathered rows: for each
    128-row chunk, lhsT is a [128, 32] "indicator" matrix whose column b
    holds 1/count[b] at rows belonging to sample b (0 elsewhere, 0 for
    pad rows). PSUM accumulates pooled [32, 768] across all chunks.
  - Classifier (768 -> 2) runs on the vector engine as two
    multiply+reduce ops against replicated weight rows, plus bias.
    Logits [32, 2] DMA'd out per core.
  - Host computes the scalar CE loss from the gathered [256, 2] logits
    (trivial epilogue) and returns (loss, logits) like the reference.

All host-side prep derives only from the tiny [256, 512] bool mask; the
384 MB of hidden states is touched exclusively by the device.
"""

import numpy as np

_B, _S, _D, _C = 256, 512, 768, 2
_NCORES = 8
_BPC = _B // _NCORES        # samples per core = 32
_P = 128                    # SBUF partitions
_SIZES = (4, 2, 1)          # gather run lengths (rows per descriptor)
_IND_SPLIT = 4              # indicator table loaded as this many tiles

LAST_RESULTS = None


def _issue_order(n_dmas):
    """Round-robin DMA issue order across run sizes: [(u, g), ...]."""
    order = []
    cnt = {u: 0 for u in _SIZES}
    while any(cnt[u] < n_dmas[u] for u in _SIZES):
        for u in _SIZES:
            if cnt[u] < n_dmas[u]:
                order.append((u, cnt[u]))
                cnt[u] += 1
    return order


def _build_program(n_dmas):
    """n_dmas: dict size -> number of gather DMAs of that run size."""
    from contextlib import ExitStack

    import concourse.bacc as bacc
    import concourse.bass as bass
    import concourse.mybir as mybir
    import concourse.tile as tile

    f32 = mybir.dt.float32
    f32r = mybir.dt.float32r
    i32 = mybir.dt.int32
    order = _issue_order(n_dmas)
    nch = sum(u for u, _ in order)
    csz = -(-nch // _IND_SPLIT)  # chunks per indicator tile

    nc = bacc.Bacc("TRN2", target_bir_lowering=False, debug=False,
                   num_devices=_NCORES)

    h_d = nc.dram_tensor("h", [_BPC * _S, _D], f32r, kind="ExternalInput")
    ix_d = {u: nc.dram_tensor(f"ix{u}", [_P, max(n_dmas[u], 1)], i32,
                              kind="ExternalInput") for u in _SIZES}
    ind_d = nc.dram_tensor("ind", [_P, nch * _BPC], f32r,
                           kind="ExternalInput")
    w_d = nc.dram_tensor("wrep", [_BPC, _C * _D], f32, kind="ExternalInput")
    b_d = nc.dram_tensor("brep", [_BPC, _C], f32, kind="ExternalInput")
    lg_d = nc.dram_tensor("logits", [_BPC, _C], f32, kind="ExternalOutput")

    with tile.TileContext(nc) as tc, ExitStack() as ctx:
        pools = {
            4: ctx.enter_context(tc.tile_pool(name="h4", bufs=6)),
            2: ctx.enter_context(tc.tile_pool(name="h2", bufs=10)),
            1: ctx.enter_context(tc.tile_pool(name="h1", bufs=12)),
        }
        cpool = ctx.enter_context(tc.tile_pool(name="const", bufs=1))
        pspool = ctx.enter_context(tc.tile_pool(name="ps", bufs=1,
                                                space="PSUM"))
        epool = ctx.enter_context(tc.tile_pool(name="ep", bufs=1))

        ix_sb = {}
        for u in _SIZES:
            ix_sb[u] = cpool.tile([_P, max(n_dmas[u], 1)], i32,
                                  name=f"ixsb{u}")
            nc.sync.dma_start(ix_sb[u][:], ix_d[u].ap())
        ind_sb = []
        for t in range(_IND_SPLIT):
            lo = t * csz * _BPC
            hi = min(nch, (t + 1) * csz) * _BPC
            tl = cpool.tile([_P, max(hi - lo, _BPC)], f32r, name=f"indsb{t}")
            if hi > lo:
                nc.sync.dma_start(tl[:, :hi - lo], ind_d.ap()[:, lo:hi])
            ind_sb.append(tl)
        wrep = cpool.tile([_BPC, _C * _D], f32)
        nc.sync.dma_start(wrep[:], w_d.ap())
        brep = cpool.tile([_BPC, _C], f32)
        nc.sync.dma_start(brep[:], b_d.ap())

        def ind_col(k):
            t, r = k // csz, k % csz
            return ind_sb[t][:, r * _BPC:(r + 1) * _BPC]

        # two accumulation epochs: epoch A's classifier reduce runs while
        # epoch B is still gathering, shrinking the kernel tail
        split = max(1, (2 * nch) // 3)
        pooled_a = pspool.tile([_BPC, _D], f32, name="pooled_a")
        pooled_b = pspool.tile([_BPC, _D], f32, name="pooled_b")
        half = {}  # (epoch, class) -> [32, 1] partial logits
        k = 0

        def classify(pooled_t, ep):
            for ci in range(_C):
                prod = epool.tile([_BPC, _D], f32, name=f"prod{ep}{ci}",
                                  tag=f"prod{ci}")
                nc.vector.tensor_mul(prod[:], pooled_t[:],
                                     wrep[:, ci * _D:(ci + 1) * _D])
                r = epool.tile([_BPC, 1], f32, name=f"r{ep}{ci}",
                               tag=f"r{ep}{ci}")
                nc.vector.reduce_sum(r[:], prod[:],
                                     axis=mybir.AxisListType.X)
                half[(ep, ci)] = r

        for u, g in order:
            ht = pools[u].tile([_P, u * _D], f32r, name=f"ht{u}",
                               tag=f"ht{u}")
            nc.gpsimd.indirect_dma_start(
                out=ht[:], out_offset=None, in_=h_d.ap()[:],
                in_offset=bass.IndirectOffsetOnAxis(
                    ap=ix_sb[u][:, g:g + 1], axis=0))
            for c in range(u):
                lhsT = ind_col(k)
                pooled = pooled_a if k < split else pooled_b
                first = k == 0 or k == split
                last = k == split - 1 or k == nch - 1
                nc.tensor.matmul(pooled[:, 0:512], lhsT,
                                 ht[:, c * _D:c * _D + 512],
                                 start=first, stop=last)
                nc.tensor.matmul(pooled[:, 512:_D], lhsT,
                                 ht[:, c * _D + 512:(c + 1) * _D],
                                 start=first, stop=last)
                k += 1
                if k == split:
                    classify(pooled_a, 0)
        classify(pooled_b, 1)

        lg = epool.tile([_BPC, _C], f32)
        for ci in range(_C):
            nc.vector.tensor_add(lg[:, ci:ci + 1], half[(0, ci)][:],
                                 half[(1, ci)][:])
        nc.vector.tensor_add(lg[:], lg[:], brep[:])
        nc.sync.dma_start(lg_d.ap(), lg[:])

    nc.compile()
    return nc


def _decompose_runs(mask_row):
    """Masked positions of one sample -> dict size -> list of run starts."""
    s = np.flatnonzero(mask_row)
    out = {u: [] for u in _SIZES}
    i = 0
    n = len(s)
    while i < n:
        j = i
        while j + 1 < n and s[j + 1] == s[j] + 1:
            j += 1
        L = j - i + 1  # maximal run s[i..j]
        pos = s[i]
        for u in _SIZES:
            while L >= u:
                out[u].append(pos)
                pos += u
                L -= u
        i = j + 1
    return out


def _prep(h, mask, W, bias):
    """Returns (n_dmas, in_maps)."""
    maskf = mask.astype(np.float32)
    counts = maskf.sum(axis=1)                      # [B]

    runs = [_decompose_runs(mask[b]) for b in range(_B)]

    # per-core dense unit lists per size: (row, owner) in sample order
    units = {}
    for core in range(_NCORES):
        b0 = core * _BPC
        for u in _SIZES:
            rows, owners = [], []
            for b in range(_BPC):
                for st in runs[b0 + b][u]:
                    rows.append(b * _S + st)
                    owners.append(b)
            units[(core, u)] = (np.asarray(rows, np.int32),
                               np.asarray(owners, np.int64))

    n_dmas = {u: max(1, max(-(-len(units[(c, u)][0]) // _P)
                            for c in range(_NCORES))) for u in _SIZES}
    order = _issue_order(n_dmas)
    nch = sum(u for u, _ in order)

    # chunk base index of each DMA in issue order
    cbase = {}
    k = 0
    for u, g in order:
        cbase[(u, g)] = k
        k += u

    wrep_np = np.ascontiguousarray(
        np.broadcast_to(W.reshape(1, _C * _D), (_BPC, _C * _D)))
    brep_np = np.ascontiguousarray(
        np.broadcast_to(bias.reshape(1, _C), (_BPC, _C)))

    in_maps = []
    for core in range(_NCORES):
        b0 = core * _BPC
        hc = np.ascontiguousarray(h[b0:b0 + _BPC].reshape(_BPC * _S, _D))
        im = {"h": hc, "wrep": wrep_np, "brep": brep_np}
        ind_np = np.zeros((_P, nch * _BPC), np.float32)
        for u in _SIZES:
            rows, owners = units[(core, u)]
            n = n_dmas[u]
            ix_np = np.zeros((_P, n), np.int32)
            nu = len(rows)
            if nu:
                w = (1.0 / counts[b0 + owners]).astype(np.float32)
                for i in range(nu):
                    g, p = divmod(i, _P)
                    ix_np[p, g] = rows[i]
                    kb = cbase[(u, g)]
                    for c in range(u):
                        ind_np[p, (kb + c) * _BPC + owners[i]] = w[i]
            im[f"ix{u}"] = np.ascontiguousarray(ix_np)
        im["ind"] = ind_np
        in_maps.append(im)
    return n_dmas, in_maps


def kernel(last_hidden_state, metaphor_mask, labels, classifier_w,
           classifier_b):
    global LAST_RESULTS
    from concourse.bass_utils import run_bass_kernel_spmd

    h = np.asarray(last_hidden_state, dtype=np.float32)
    mask = np.asarray(metaphor_mask).astype(bool)
    labels = np.asarray(labels)
    W = np.asarray(classifier_w, dtype=np.float32)
    bias = np.asarray(classifier_b, dtype=np.float32)

    assert h.shape == (_B, _S, _D) and W.shape == (_C, _D)

    n_dmas, in_maps = _prep(h, mask, W, bias)
    nc = _build_program(n_dmas)

    res = run_bass_kernel_spmd(nc, in_maps, core_ids=list(range(_NCORES)))
    LAST_RESULTS = res
    logits = np.concatenate([r["logits"] for r in res.results], axis=0)

    # Host epilogue: CE loss (mean reduction) over the tiny [256, 2] logits.
    lg64 = logits.astype(np.float64)
    m = lg64.max(axis=1, keepdims=True)
    lse = (m[:, 0] + np.log(np.exp(lg64 - m).sum(axis=1)))
    nll = lse - lg64[np.arange(_B), labels.astype(np.int64)]
    loss = np.float32(nll.mean())
    return loss, logits
